# revision 11
# baseline (speedup 1.0000x reference)
"""Trainium2 Bass kernel for nn_Aligner (location-sensitive attention + GRU scan).

Sharding: data-parallel over batch across 8 NeuronCores (4 utterances/core),
weights replicated; each core runs the full sequential T-step scan.

Key layout decisions (per core, BL=4 local utterances):
 - All big per-step GEMMs run with the small batch as the *stationary* PE
   operand and the bf16 weights as the *moving* operand (128 elem/cycle), in
   [b, feat] output orientation; N=512 chunks (one PSUM bank per matmul).
 - h / ctx / alpha are re-transposed to lhsT layout each step via PE
   transpose-mode (tiny [4,128] tiles).
 - score lives as [c2-chunk partitions, (b,s) free]; tanh is one big ACT op;
   the w_agg contraction is a K=128 matmul chain; softmax runs in [1, (b,s)]
   then bounces through DRAM into [4, S] row layout for the alpha recursion.
 - conv1d(align) is composed host-side into a single [C2, 31] weight; its
   im2col A_shift matrix is built with one overlapping-window DMA from a
   zero-padded DRAM buffer.
 - The t-branch (trans scalar) only feeds the *next* step, so it runs at the
   tail. attend_t == prev_{t+1} (same einsum), computed once as ctx.

PSUM (8 banks) is tag-shared: slot A (4 banks): gx_rz -> score passes;
slot B (4 banks): gn2(xn,ghn) -> hT -> qp -> energy -> aT -> ctx -> ctxT -> t1.
"""

import sys
import numpy as np

sys.path.insert(0, "/opt/trn_rl_repo")

import bass_rust
from concourse import bass, bacc, tile
import concourse.mybir as mybir
from concourse.bass_utils import run_bass_kernel_spmd

F32 = mybir.dt.float32
BF16 = mybir.dt.bfloat16
AF = mybir.ActivationFunctionType
ALU = mybir.AluOpType
PE = mybir.EngineType.PE

B_FULL, S, T_FULL = 32, 256, 800
I, H, M, C, LOC, KC = 512, 256, 80, 1024, 32, 31
C2 = C // 2
NCORES = 8
BL = B_FULL // NCORES          # 4
G3 = 3 * C                     # 3072
PAD = S + 30                   # 286


def _ap(handle_or_ap, steps_counts, offset=0):
    """Raw [step,count] access pattern over a tensor (element units)."""
    if isinstance(handle_or_ap, bass_rust.AP):
        ap = handle_or_ap.copy()
    else:
        ap = handle_or_ap.ap().copy()
    ap.ap = bass_rust.VecI64Pair(steps_counts)
    ap.offset = offset
    return ap


def build_program(T):
    nc = bacc.Bacc("TRN2", target_bir_lowering=False, debug=False)
    R = BL * T

    def din(name, shape, dt):
        return nc.dram_tensor(name, list(shape), dt, kind="ExternalInput")

    enc_bf_d = din("enc_bf", [128, 8, I], BF16)
    encT_d = din("encT", [128, 4, BL * S], F32)
    qT_d = din("qT", [128, 2, R], BF16)
    frT_d = din("frT", [BL * T, M // BL, BL], BF16)   # rows t*4+q, [20,4]
    wihp_d = din("wihp", [128, 4, G3], BF16)
    whh_d = din("whh", [128, 8, G3], BF16)
    wq_d = din("wq", [128, 8, C2], BF16)
    wt1a_d = din("wt1a", [128, 4, C], BF16)
    wt1h_d = din("wt1h", [128, 8, C], BF16)
    wt1f_d = din("wt1f", [M, C], BF16)
    wihq_d = din("wihq", [128, 2, G3], BF16)
    wk_d = din("wk", [128, 4, C2], F32)
    weff_d = din("weff", [KC, C2], F32)
    wagg_d = din("wagg", [128, 4], BF16)
    bk_d = din("bk", [128, 4], F32)
    bias1_d = din("bias1", [1, G3], F32)
    bhhn_d = din("bhhn", [1, C], F32)
    bt1_d = din("bt1", [1, C], BF16)
    bsel_d = din("bsel", [BL, BL * S], F32)
    i4bf_d = din("i4bf", [BL, BL], BF16)
    ones4bf_d = din("ones4bf", [1, BL], BF16)
    id4_d = din("id4", [BL, BL], F32)
    ones1_d = din("ones1", [1, 128], F32)
    wt2r_d = din("wt2r", [BL, C], F32)

    alphas_d = nc.dram_tensor("alphas", [R, S], F32, kind="ExternalOutput")

    gq_d = nc.dram_tensor("gq_scratch", [R, G3], BF16)
    eb_d = nc.dram_tensor("e_bounce", [BL * S], F32)
    apd = nc.dram_tensor("align_pad", [BL, PAD], F32)

    with tile.TileContext(nc) as tc:
        with (
            tc.tile_pool(name="const", bufs=1) as cpool,
            tc.tile_pool(name="state", bufs=1) as spool,
            tc.tile_pool(name="work", bufs=1) as wpool,
            tc.tile_pool(name="psum", bufs=1, space="PSUM") as ppool,
        ):
            def load(dram, shape, dt, tag):
                t = cpool.tile(list(shape), dt, tag=tag)
                nc.sync.dma_start(t[:], dram.ap())
                return t

            ones1 = load(ones1_d, [1, 128], F32, "ones1")
            key_sb = cpool.tile([128, 4, BL * S], BF16, tag="key_sb")

            # ===== precompute (aliased into const slots, loaded later) =====
            if True:
                encT = cpool.tile([128, 4, BL * S], F32, tag="whh")
                nc.sync.dma_start(encT[:], encT_d.ap())
                wk = cpool.tile([128, 4, C2], F32, tag="wq")
                nc.sync.dma_start(wk[:], wk_d.ap())
                wihq = cpool.tile([128, 2, G3], BF16, tag="wihp")
                nc.sync.dma_start(wihq[:], wihq_d.ap())
                bias1 = cpool.tile([1, G3], F32, tag="wt1h")
                nc.sync.dma_start(bias1[:], bias1_d.ap())

                # key[c2chunk, (b,s)] = w_k.T^T @ encT   (fp32)
                for mc in range(4):
                    kps = ppool.tile([128, BL * S], F32, tag="pB")
                    for nk in range(2):
                        for kc in range(4):
                            nc.tensor.matmul(
                                kps[:, nk * 512:(nk + 1) * 512],
                                wk[:, kc, mc * 128:(mc + 1) * 128],
                                encT[:, kc, nk * 512:(nk + 1) * 512],
                                start=(kc == 0), stop=(kc == 3))
                    nc.vector.tensor_copy(key_sb[:, mc, :], kps[:])  # f32->bf16 cast

                # gq rows: q @ w_ih_q.T + bias1 -> bf16 DRAM
                nmc = (R + 127) // 128
                for mc in range(nmc):
                    r0 = mc * 128
                    rr = min(128, R - r0)
                    qts = cpool.tile([128, 2, 128], BF16, tag="enc_bf",
                                     bufs=1)
                    nc.sync.dma_start(qts[:, :, :rr],
                                      qT_d.ap()[:, :, r0:r0 + rr])
                    for half in range(2):
                        gps = ppool.tile([128, G3 // 2], F32, tag="pA")
                        for nk in range(3):
                            col = (half * 3 + nk) * 512
                            for kc in range(2):
                                nc.tensor.matmul(
                                    gps[:rr, nk * 512:(nk + 1) * 512],
                                    qts[:, kc, :rr],
                                    wihq[:, kc, col:col + 512],
                                    start=(kc == 0), stop=False)
                            nc.tensor.matmul(
                                gps[:rr, nk * 512:(nk + 1) * 512],
                                ones1[:, :rr],
                                bias1[:, col:col + 512],
                                start=False, stop=True)
                        gsb = cpool.tile([128, G3 // 2], BF16, tag="wt1a",
                                         bufs=1)
                        if half == 0:
                            nc.vector.tensor_copy(gsb[:rr, :], gps[:rr, :])
                        else:
                            nc.scalar.copy(gsb[:rr, :], gps[:rr, :])
                        nc.sync.dma_start(
                            gq_d.ap()[r0:r0 + rr,
                                      half * (G3 // 2):(half + 1) * (G3 // 2)],
                            gsb[:rr, :])

            # big constants loaded after the precompute pool is closed
            enc_bf = load(enc_bf_d, [128, 8, I], BF16, "enc_bf")
            wihp = load(wihp_d, [128, 4, G3], BF16, "wihp")
            whh = load(whh_d, [128, 8, G3], BF16, "whh")
            wq = load(wq_d, [128, 8, C2], BF16, "wq")
            wt1a = load(wt1a_d, [128, 4, C], BF16, "wt1a")
            wt1h = load(wt1h_d, [128, 8, C], BF16, "wt1h")
            wt1f = load(wt1f_d, [M, C], BF16, "wt1f")
            weff = load(weff_d, [KC, C2], F32, "weff")
            wagg = load(wagg_d, [128, 4], BF16, "wagg")
            bk = load(bk_d, [128, 4], F32, "bk")
            bhhn = load(bhhn_d, [1, C], F32, "bhhn")
            bt1 = load(bt1_d, [1, C], BF16, "bt1")
            bsel = load(bsel_d, [BL, BL * S], F32, "bsel")
            i4bf = load(i4bf_d, [BL, BL], BF16, "i4bf")
            ones4bf = load(ones4bf_d, [1, BL], BF16, "ones4bf")
            id4 = load(id4_d, [BL, BL], F32, "id4")
            wt2r = load(wt2r_d, [BL, C], F32, "wt2r")

            # ================= state =================
            h_b = spool.tile([BL, C], F32)
            hT = spool.tile([128, 8, BL], BF16)
            ctxT = spool.tile([128, 4, BL], BF16)
            alf = spool.tile([BL, S + 1], F32)
            trans = spool.tile([BL, 1], F32)
            aD = spool.tile([128, 8, BL], BF16)
            ash = spool.tile([KC, BL * S], F32)
            alsc = spool.tile([BL, S], F32)

            nc.gpsimd.memset(h_b[:], 0.0)
            nc.gpsimd.memset(hT[:], 0.0)
            nc.gpsimd.memset(alf[:], 0.0)
            nc.gpsimd.memset(trans[:], 0.5)
            nc.gpsimd.memset(aD[:], 0.0)
            nc.gpsimd.memset(ash[:], 0.0)
            nc.sync.dma_start(apd.ap()[:, :], ash[0:BL, 0:PAD])
            nc.gpsimd.memset(alf[:, 1:2], 1.0)
            for b in range(BL):
                nc.gpsimd.memset(aD[0:1, 2 * b, b:b + 1], 1.0)
            nc.gpsimd.memset(alsc[:], 1.0 / S)
            nc.sync.dma_start(apd.ap()[:, 15:15 + S], alsc[:])
            nc.sync.dma_start(ash[:, :],
                              _ap(apd, [[1, KC], [PAD, BL], [1, S]]))

            def ctx_block():
                cps = ppool.tile([BL, I], F32, tag="pB")
                for kc in range(8):
                    nc.tensor.matmul(cps[:], aD[:, kc, :], enc_bf[:, kc, :],
                                     start=(kc == 0), stop=(kc == 7))
                ctx_b = wpool.tile([BL, I], F32, tag="ctxb")
                nc.vector.tensor_copy(ctx_b[:], cps[:])
                tps = ppool.tile([128, 4, BL], F32, tag="pB")
                for ck in range(4):
                    nc.tensor.transpose(
                        tps[:, ck, :], ctx_b[:, ck * 128:(ck + 1) * 128],
                        id4[:])
                nc.vector.tensor_copy(ctxT[:], tps[:])

            ctx_block()

            # ================= scan =================
            with tc.For_i(0, R, BL, hint_engines=(PE,)) as iv:
                # ---- gate preactivations ----
                # gx_rz: [4, 2048] = (gq+bias) + gprev + gh  (r,z cols)
                # gn2:   [4, 2048] = [xn | ghn]
                gxrz = ppool.tile([BL, 2 * C], F32, tag="pA")
                gn2 = ppool.tile([BL, 2 * C], F32, tag="pB")

                gq_sb = wpool.tile([BL, G3], BF16, tag="gqstep")
                nc.sync.dma_start(gq_sb[:], gq_d.ap()[bass.ds(iv, BL), :])

                def gdst(nk):
                    if nk < 4:
                        return gxrz[:, nk * 512:(nk + 1) * 512]
                    return gn2[:, (nk - 2) * 512:(nk - 1) * 512]  # ghn half

                for kc in range(8):  # gh
                    for nk in range(6):
                        nc.tensor.matmul(
                            gdst(nk), hT[:, kc, :],
                            whh[:, kc, nk * 512:(nk + 1) * 512],
                            start=(kc == 0), stop=False)
                for nk in range(2):  # b_hh_n closes ghn
                    nc.tensor.matmul(
                        gn2[:, (nk + 2) * 512:(nk + 3) * 512],
                        ones1[:, :BL],
                        bhhn[:, nk * 512:(nk + 1) * 512],
                        start=False, stop=True)

                def xdst(nk):
                    if nk < 4:
                        return gxrz[:, nk * 512:(nk + 1) * 512]
                    return gn2[:, (nk - 4) * 512:(nk - 3) * 512]  # xn half

                for kc in range(4):  # gprev
                    for nk in range(6):
                        nc.tensor.matmul(
                            xdst(nk), ctxT[:, kc, :],
                            wihp[:, kc, nk * 512:(nk + 1) * 512],
                            start=(kc == 0 and nk >= 4), stop=False)
                for nk in range(6):  # gq identity-add closes gxrz + xn
                    nc.tensor.matmul(
                        xdst(nk), i4bf[:],
                        gq_sb[:, nk * 512:(nk + 1) * 512],
                        start=False, stop=True)

                # ---- gates / GRU update (row layout) ----
                trz = wpool.tile([BL, 2 * C], F32, tag="trz")
                nc.scalar.activation(trz[:], gxrz[:], AF.Tanh, scale=0.5)
                hn05 = wpool.tile([BL, C], F32, tag="gtmp", bufs=2)
                nc.scalar.activation(hn05[:], gn2[:, C:], AF.Copy, scale=0.5)
                o2 = wpool.tile([BL, C], F32, tag="gtmp", bufs=2)
                nc.vector.scalar_tensor_tensor(
                    o2[:], trz[:, :C], 1.0, hn05[:], ALU.add, ALU.mult)
                narg = wpool.tile([BL, C], F32, tag="gtmp", bufs=2)
                nc.vector.tensor_add(narg[:], gn2[:, :C], o2[:])
                ngate = wpool.tile([BL, C], F32, tag="ngate")
                nc.scalar.activation(ngate[:], narg[:], AF.Tanh)
                dmn = wpool.tile([BL, C], F32, tag="gtmp", bufs=2)
                nc.vector.tensor_sub(dmn[:], h_b[:], ngate[:])
                o5 = wpool.tile([BL, C], F32, tag="gtmp", bufs=2)
                nc.vector.scalar_tensor_tensor(
                    o5[:], trz[:, C:], 1.0, dmn[:], ALU.add, ALU.mult)
                nc.vector.scalar_tensor_tensor(
                    h_b[:], o5[:], 0.5, ngate[:], ALU.mult, ALU.add)

                hps = ppool.tile([128, 8, BL], F32, tag="pB")
                for ck in range(8):
                    nc.tensor.transpose(
                        hps[:, ck, :], h_b[:, ck * 128:(ck + 1) * 128], id4[:])
                nc.vector.tensor_copy(hT[:], hps[:])

                # ---- qp = h_new @ w_q.T ----
                qps = ppool.tile([BL, C2], F32, tag="pB")
                for kc in range(8):
                    nc.tensor.matmul(qps[:], hT[:, kc, :], wq[:, kc, :],
                                     start=(kc == 0), stop=(kc == 7))
                qp_sb = wpool.tile([BL, C2], F32, tag="qpsb")
                nc.vector.tensor_copy(qp_sb[:], qps[:])

                # ---- score (two 2-chunk passes) + tanh + energy ----
                eps = ppool.tile([1, BL * S], F32, tag="pB")
                for hp in range(2):
                    scps = ppool.tile([128, 2, BL * S], F32, tag="pA")
                    for m2 in range(2):
                        mc = hp * 2 + m2
                        for nk in range(2):
                            sl = scps[:, m2, nk * 512:(nk + 1) * 512]
                            nc.tensor.matmul(
                                sl, weff[:, mc * 128:(mc + 1) * 128],
                                ash[:, nk * 512:(nk + 1) * 512],
                                start=True, stop=False)
                            nc.tensor.matmul(
                                sl, qp_sb[:, mc * 128:(mc + 1) * 128],
                                bsel[:, nk * 512:(nk + 1) * 512],
                                start=False, stop=True)
                    for m2 in range(2):
                        mc = hp * 2 + m2
                        ssbt = wpool.tile([128, BL * S], F32, tag="ssbt",
                                          bufs=1)
                        nc.vector.scalar_tensor_tensor(
                            ssbt[:], scps[:, m2, :], bk[:, mc:mc + 1],
                            key_sb[:, mc, :], ALU.add, ALU.add)
                        taut = wpool.tile([128, BL * S], BF16, tag="taut",
                                          bufs=1)
                        nc.scalar.activation(taut[:], ssbt[:], AF.Tanh)
                        for nk in range(2):
                            nc.tensor.matmul(
                                eps[:, nk * 512:(nk + 1) * 512],
                                wagg[:, mc:mc + 1],
                                taut[:, nk * 512:(nk + 1) * 512],
                                start=(mc == 0), stop=(mc == 3))

                # ---- softmax / alpha recursion ----
                erow = wpool.tile([1, BL * S], F32, tag="ssbt", bufs=1)
                nc.scalar.activation(erow[:], eps[:], AF.Exp)
                nc.sync.dma_start(eb_d.ap()[:], erow[:])
                e4 = wpool.tile([BL, S], F32, tag="e4")
                nc.sync.dma_start(e4[:], _ap(eb_d, [[S, BL], [1, S]]))

                omt = wpool.tile([BL, 1], F32, tag="omt")
                nc.vector.tensor_scalar(omt[:], trans[:], -1.0, 1.0,
                                        ALU.mult, ALU.add)
                m1 = wpool.tile([BL, S], F32, tag="al", bufs=2)
                nc.vector.tensor_scalar(m1[:], alf[:, 1:], omt[:], 1e-7,
                                        ALU.mult, ALU.add)
                mix = wpool.tile([BL, S], F32, tag="al", bufs=2)
                nc.vector.scalar_tensor_tensor(
                    mix[:], alf[:, 0:S], trans[:], m1[:], ALU.mult, ALU.add)
                u = wpool.tile([BL, S], F32, tag="al", bufs=2)
                nc.vector.tensor_mul(u[:], mix[:], e4[:])
                usum = wpool.tile([BL, 1], F32, tag="usum")
                nc.vector.reduce_sum(usum[:], u[:], mybir.AxisListType.X)
                urec = wpool.tile([BL, 1], F32, tag="urec")
                nc.vector.reciprocal(urec[:], usum[:])
                nc.vector.tensor_scalar(alf[:, 1:], u[:], urec[:], None,
                                        ALU.mult)
                nc.sync.dma_start(alphas_d.ap()[bass.ds(iv, BL), :],
                                  alf[:, 1:])

                # align for next step's conv
                zs = wpool.tile([BL, 1], F32, tag="zs")
                nc.vector.reduce_sum(zs[:], e4[:], mybir.AxisListType.X)
                zr = wpool.tile([BL, 1], F32, tag="zr")
                nc.vector.reciprocal(zr[:], zs[:])
                nc.vector.tensor_scalar(alsc[:], e4[:], zr[:], None, ALU.mult)
                nc.sync.dma_start(apd.ap()[:, 15:15 + S], alsc[:])
                nc.sync.dma_start(ash[:, :],
                                  _ap(apd, [[1, KC], [PAD, BL], [1, S]]))

                # ---- alpha -> aD (block diagonal, bf16) ----
                aps = ppool.tile([128, 2, BL], F32, tag="pB")
                nc.tensor.transpose(aps[:, 0, :], alf[:, 1:129], id4[:])
                nc.tensor.transpose(aps[:, 1, :], alf[:, 129:257], id4[:])
                for seg in range(2):
                    dst = _ap(aD[:], [[8 * BL, 128], [2 * BL + 1, BL]],
                              BL * seg)
                    nc.vector.tensor_copy(dst, aps[:, seg, :])

                # ---- ctx (= attend_t = prev_{t+1}) ----
                ctx_block()

                # ---- t-branch: trans_{t+1} ----
                t1p = ppool.tile([BL, C], F32, tag="pB")
                for nk in range(2):
                    cs = slice(nk * 512, (nk + 1) * 512)
                    for kc in range(4):
                        nc.tensor.matmul(
                            t1p[:, cs], ctxT[:, kc, :], wt1a[:, kc, cs],
                            start=(kc == 0), stop=False)
                    for kc in range(8):
                        nc.tensor.matmul(
                            t1p[:, cs], hT[:, kc, :], wt1h[:, kc, cs],
                            start=False, stop=False)
                frt = wpool.tile([M, BL], BF16, tag="frt")
                nc.sync.dma_start(frt[:], frT_d.ap()[bass.ds(iv, BL), :, :])
                for nk in range(2):
                    cs = slice(nk * 512, (nk + 1) * 512)
                    nc.tensor.matmul(t1p[:, cs], frt[:], wt1f[:, cs],
                                     start=False, stop=False)
                    nc.tensor.matmul(t1p[:, cs], ones4bf[:], bt1[:, cs],
                                     start=False, stop=True)
                tt1 = wpool.tile([BL, C], F32, tag="gtmp", bufs=2)
                nc.scalar.activation(tt1[:], t1p[:], AF.Tanh)
                tu = wpool.tile([BL, C], F32, tag="gtmp", bufs=2)
                nc.vector.tensor_mul(tu[:], tt1[:], wt2r[:])
                ts = wpool.tile([BL, 1], F32, tag="ts")
                nc.vector.reduce_sum(ts[:], tu[:], mybir.AxisListType.X)
                tt = wpool.tile([BL, 1], F32, tag="tt")
                nc.scalar.activation(tt[:], ts[:], AF.Tanh, scale=0.5)
                nc.vector.tensor_scalar(trans[:], tt[:], 0.5, 0.5,
                                        ALU.mult, ALU.add)

    return nc


def _prep_shared(inputs):
    w_ih = np.asarray(inputs["w_ih"], np.float32)
    w_hh = np.asarray(inputs["w_hh"], np.float32)
    b_ih = np.asarray(inputs["b_ih"], np.float32)
    b_hh = np.asarray(inputs["b_hh"], np.float32)
    w_q = np.asarray(inputs["w_q"], np.float32)
    w_loc1 = np.asarray(inputs["w_loc1"], np.float32)
    w_loc2 = np.asarray(inputs["w_loc2"], np.float32)
    w_k = np.asarray(inputs["w_k"], np.float32)
    b_k = np.asarray(inputs["b_k"], np.float32)
    w_agg = np.asarray(inputs["w_agg"], np.float32)
    w_t1 = np.asarray(inputs["w_t1"], np.float32)
    b_t1 = np.asarray(inputs["b_t1"], np.float32)
    w_t2 = np.asarray(inputs["w_t2"], np.float32)

    w_eff = w_loc2 @ w_loc1[:, 0, :]  # [C2, KC]
    bias1 = b_ih + np.concatenate([b_hh[:2 * C], np.zeros(C, np.float32)])
    bsel = np.zeros((BL, BL * S), np.float32)
    for b in range(BL):
        bsel[b, b * S:(b + 1) * S] = 1.0

    cc = np.ascontiguousarray

    def chunk(a):  # [nk*128, X] -> [128, nk, X]
        nk = a.shape[0] // 128
        return cc(a.reshape(nk, 128, -1).transpose(1, 0, 2))

    return {
        "wihp": chunk(w_ih[:, H:].T),
        "whh": chunk(w_hh.T),
        "wq": chunk(w_q.T),
        "wt1a": chunk(w_t1[:, :I].T),
        "wt1h": chunk(w_t1[:, I + M:].T),
        "wt1f": cc(w_t1[:, I:I + M].T),
        "wihq": chunk(w_ih[:, :H].T),
        "wk": chunk(w_k.T),
        "weff": cc(w_eff.T),
        "wagg": cc(w_agg.reshape(4, 128).T),
        "bk": cc(b_k.reshape(4, 128).T),
        "bias1": bias1.reshape(1, G3),
        "bhhn": cc(b_hh[2 * C:].reshape(1, C)),
        "bt1": cc(b_t1.reshape(1, C)),
        "bsel": bsel,
        "i4bf": np.eye(BL, dtype=np.float32),
        "ones4bf": np.ones((1, BL), np.float32),
        "id4": np.eye(BL, dtype=np.float32),
        "ones1": np.ones((1, 128), np.float32),
        "wt2r": np.tile(w_t2.reshape(1, C), (BL, 1)),
    }


_BF16_NAMES = {"enc_bf", "qT", "frT", "wihp", "whh", "wq", "wt1a", "wt1h",
               "wt1f", "wihq", "wagg", "bt1", "i4bf", "ones4bf"}


def make_in_maps(inputs):
    import ml_dtypes

    def cast(name, arr):
        if name in _BF16_NAMES:
            return np.asarray(arr, np.float32).astype(ml_dtypes.bfloat16)
        return np.ascontiguousarray(arr, np.float32)

    T = inputs["queries"].shape[1]
    shared = _prep_shared(inputs)
    enc = np.asarray(inputs["encodings"], np.float32)
    qs = np.asarray(inputs["queries"], np.float32)
    outs = np.asarray(inputs["outputs"], np.float32)

    in_maps = []
    for c in range(NCORES):
        sl = slice(c * BL, (c + 1) * BL)
        e = enc[sl].reshape(BL * S, I)
        q = qs[sl]
        fr = outs[sl]
        m = {k: cast(k, v) for k, v in shared.items()}
        m["enc_bf"] = cast("enc_bf", e.reshape(8, 128, I).transpose(1, 0, 2))
        m["encT"] = cast("encT", e.T.reshape(4, 128, BL * S).transpose(1, 0, 2))
        m["qT"] = cast("qT", q.transpose(2, 1, 0).reshape(
            2, 128, T * BL).transpose(1, 0, 2))
        m["frT"] = cast("frT", fr.transpose(1, 2, 0).reshape(
            T, BL, M // BL, BL).reshape(T * BL, M // BL, BL))
        in_maps.append(m)
    return in_maps


def kernel(**inputs):
    mask = np.asarray(inputs["mask"])
    assert np.all(mask == 1.0), "kernel assumes all-ones mask"
    T = inputs["queries"].shape[1]

    import os, time as _time
    in_maps = make_in_maps(inputs)
    nc = build_program(T)
    nc.compile()
    t0 = _time.time()
    res = run_bass_kernel_spmd(nc, in_maps, list(range(NCORES)))
    if os.environ.get("ALIGNER_BENCH"):
        print(f"exec+jit wall: {_time.time()-t0:.2f}s", flush=True)
        for it in range(2):
            t0 = _time.time()
            res = run_bass_kernel_spmd(nc, in_maps, list(range(NCORES)))
            w = _time.time() - t0
            print(f"exec wall[{it}]: {w:.3f}s  HW exec time: {w*1e9:.0f} ns",
                  flush=True)
    out = np.zeros((B_FULL, T, S), np.float32)
    for c in range(NCORES):
        a = np.asarray(res.results[c]["alphas"], np.float32).reshape(T, BL, S)
        out[c * BL:(c + 1) * BL] = a.transpose(1, 0, 2)
    return out


if __name__ == "__main__":
    build_program(2)
    print("build ok")


# revision 13
# speedup vs baseline: 1.1838x; 1.1838x over previous
"""Trainium2 Bass kernel for nn_Aligner (location-sensitive attention + GRU scan).

Sharding: data-parallel over batch across 8 NeuronCores (4 utterances/core),
weights replicated; each core runs the full sequential T-step scan.

Key layout decisions (per core, BL=4 local utterances):
 - All big per-step GEMMs run with the small batch as the *stationary* PE
   operand and the bf16 weights as the *moving* operand (128 elem/cycle), in
   [b, feat] output orientation; N=512 chunks (one PSUM bank per matmul).
 - h / ctx / alpha are re-transposed to lhsT layout each step via PE
   transpose-mode (tiny [4,128] tiles).
 - score lives as [c2-chunk partitions, (b,s) free]; tanh is one big ACT op;
   the w_agg contraction is a K=128 matmul chain; softmax runs in [1, (b,s)]
   then bounces through DRAM into [4, S] row layout for the alpha recursion.
 - conv1d(align) is composed host-side into a single [C2, 31] weight; its
   im2col A_shift matrix is built with one overlapping-window DMA from a
   zero-padded DRAM buffer.
 - The t-branch (trans scalar) only feeds the *next* step, so it runs at the
   tail. attend_t == prev_{t+1} (same einsum), computed once as ctx.

PSUM (8 banks) is tag-shared: slot A (4 banks): gx_rz -> score passes;
slot B (4 banks): gn2(xn,ghn) -> hT -> qp -> energy -> aT -> ctx -> ctxT -> t1.
"""

import sys
import numpy as np

sys.path.insert(0, "/opt/trn_rl_repo")

import bass_rust
from concourse import bass, bacc, tile
import concourse.mybir as mybir
from concourse.bass_utils import run_bass_kernel_spmd

F32 = mybir.dt.float32
BF16 = mybir.dt.bfloat16
AF = mybir.ActivationFunctionType
ALU = mybir.AluOpType
PE = mybir.EngineType.PE

B_FULL, S, T_FULL = 32, 256, 800
I, H, M, C, LOC, KC = 512, 256, 80, 1024, 32, 31
C2 = C // 2
NCORES = 8
BL = B_FULL // NCORES          # 4
G3 = 3 * C                     # 3072
PAD = S + 30                   # 286


def _ap(handle_or_ap, steps_counts, offset=0):
    """Raw [step,count] access pattern over a tensor (element units)."""
    if isinstance(handle_or_ap, bass_rust.AP):
        ap = handle_or_ap.copy()
    else:
        ap = handle_or_ap.ap().copy()
    ap.ap = bass_rust.VecI64Pair(steps_counts)
    ap.offset = offset
    return ap


def build_program(T):
    nc = bacc.Bacc("TRN2", target_bir_lowering=False, debug=False)
    R = BL * T

    def din(name, shape, dt):
        return nc.dram_tensor(name, list(shape), dt, kind="ExternalInput")

    enc_bf_d = din("enc_bf", [128, 8, I], BF16)
    encT_d = din("encT", [128, 4, BL * S], F32)
    qT_d = din("qT", [128, 2, R], BF16)
    frT_d = din("frT", [BL * T, M // BL, BL], BF16)   # rows t*4+q, [20,4]
    wihp_d = din("wihp", [128, 4, G3], BF16)
    whh_d = din("whh", [128, 8, G3], BF16)
    wq_d = din("wq", [128, 8, C2], BF16)
    wt1a_d = din("wt1a", [128, 4, C], BF16)
    wt1h_d = din("wt1h", [128, 8, C], BF16)
    wt1f_d = din("wt1f", [M, C], BF16)
    wihq_d = din("wihq", [128, 2, G3], BF16)
    wk_d = din("wk", [128, 4, C2], F32)
    weff_d = din("weff", [KC, C2], F32)
    wagg_d = din("wagg", [128, 4], BF16)
    bk_d = din("bk", [128, 4], F32)
    bias1_d = din("bias1", [1, G3], F32)
    bhhn_d = din("bhhn", [1, C], F32)
    bt1_d = din("bt1", [1, C], BF16)
    bsel_d = din("bsel", [BL, BL * S], F32)
    i4bf_d = din("i4bf", [BL, BL], BF16)
    ones4bf_d = din("ones4bf", [1, BL], BF16)
    id4_d = din("id4", [BL, BL], F32)
    ones1_d = din("ones1", [1, 128], F32)
    wt2r_d = din("wt2r", [BL, C], F32)

    alphas_d = nc.dram_tensor("alphas", [R, S], F32, kind="ExternalOutput")

    gq_d = nc.dram_tensor("gq_scratch", [R, G3], BF16)
    eb_d = nc.dram_tensor("e_bounce", [BL * S], F32)
    apd = nc.dram_tensor("align_pad", [BL, PAD], F32)

    with tile.TileContext(nc) as tc:
        with (
            tc.tile_pool(name="const", bufs=1) as cpool,
            tc.tile_pool(name="state", bufs=1) as spool,
            tc.tile_pool(name="work", bufs=1) as wpool,
            tc.tile_pool(name="psum", bufs=1, space="PSUM") as ppool,
        ):
            def load(dram, shape, dt, tag):
                t = cpool.tile(list(shape), dt, tag=tag)
                nc.sync.dma_start(t[:], dram.ap())
                return t

            ones1 = load(ones1_d, [1, 128], F32, "ones1")
            key_sb = cpool.tile([128, 4, BL * S], BF16, tag="key_sb")

            # ===== precompute (aliased into const slots, loaded later) =====
            if True:
                encT = cpool.tile([128, 4, BL * S], F32, tag="whh")
                nc.sync.dma_start(encT[:], encT_d.ap())
                wk = cpool.tile([128, 4, C2], F32, tag="wq")
                nc.sync.dma_start(wk[:], wk_d.ap())
                wihq = cpool.tile([128, 2, G3], BF16, tag="wihp")
                nc.sync.dma_start(wihq[:], wihq_d.ap())
                bias1 = cpool.tile([1, G3], F32, tag="wt1h")
                nc.sync.dma_start(bias1[:], bias1_d.ap())

                # key[c2chunk, (b,s)] = w_k.T^T @ encT   (fp32)
                for mc in range(4):
                    kps = ppool.tile([128, BL * S], F32, tag="pB")
                    for nk in range(2):
                        for kc in range(4):
                            nc.tensor.matmul(
                                kps[:, nk * 512:(nk + 1) * 512],
                                wk[:, kc, mc * 128:(mc + 1) * 128],
                                encT[:, kc, nk * 512:(nk + 1) * 512],
                                start=(kc == 0), stop=(kc == 3))
                    nc.vector.tensor_copy(key_sb[:, mc, :], kps[:])  # f32->bf16 cast

                # gq rows: q @ w_ih_q.T + bias1 -> bf16 DRAM
                nmc = (R + 127) // 128
                for mc in range(nmc):
                    r0 = mc * 128
                    rr = min(128, R - r0)
                    qts = cpool.tile([128, 2, 128], BF16, tag="enc_bf",
                                     bufs=1)
                    nc.sync.dma_start(qts[:, :, :rr],
                                      qT_d.ap()[:, :, r0:r0 + rr])
                    for half in range(2):
                        gps = ppool.tile([128, G3 // 2], F32, tag="pA")
                        for nk in range(3):
                            col = (half * 3 + nk) * 512
                            for kc in range(2):
                                nc.tensor.matmul(
                                    gps[:rr, nk * 512:(nk + 1) * 512],
                                    qts[:, kc, :rr],
                                    wihq[:, kc, col:col + 512],
                                    start=(kc == 0), stop=False)
                            nc.tensor.matmul(
                                gps[:rr, nk * 512:(nk + 1) * 512],
                                ones1[:, :rr],
                                bias1[:, col:col + 512],
                                start=False, stop=True)
                        gsb = cpool.tile([128, G3 // 2], BF16, tag="wt1a",
                                         bufs=1)
                        if half == 0:
                            nc.vector.tensor_copy(gsb[:rr, :], gps[:rr, :])
                        else:
                            nc.scalar.copy(gsb[:rr, :], gps[:rr, :])
                        nc.sync.dma_start(
                            gq_d.ap()[r0:r0 + rr,
                                      half * (G3 // 2):(half + 1) * (G3 // 2)],
                            gsb[:rr, :])

            # big constants loaded after the precompute pool is closed
            enc_bf = load(enc_bf_d, [128, 8, I], BF16, "enc_bf")
            wihp = load(wihp_d, [128, 4, G3], BF16, "wihp")
            whh = load(whh_d, [128, 8, G3], BF16, "whh")
            wq = load(wq_d, [128, 8, C2], BF16, "wq")
            wt1a = load(wt1a_d, [128, 4, C], BF16, "wt1a")
            wt1h = load(wt1h_d, [128, 8, C], BF16, "wt1h")
            wt1f = load(wt1f_d, [M, C], BF16, "wt1f")
            weff = load(weff_d, [KC, C2], F32, "weff")
            wagg = load(wagg_d, [128, 4], BF16, "wagg")
            bk = load(bk_d, [128, 4], F32, "bk")
            bhhn = load(bhhn_d, [1, C], F32, "bhhn")
            bt1 = load(bt1_d, [1, C], BF16, "bt1")
            bsel = load(bsel_d, [BL, BL * S], F32, "bsel")
            i4bf = load(i4bf_d, [BL, BL], BF16, "i4bf")
            ones4bf = load(ones4bf_d, [1, BL], BF16, "ones4bf")
            id4 = load(id4_d, [BL, BL], F32, "id4")
            wt2r = load(wt2r_d, [BL, C], F32, "wt2r")

            # ================= state =================
            h_b = spool.tile([BL, C], F32)
            hT = spool.tile([128, 8, BL], BF16)
            ctxT = spool.tile([128, 4, BL], BF16)
            alf = spool.tile([BL, S + 1], F32)
            trans = spool.tile([BL, 1], F32)
            aD = spool.tile([128, 8, BL], BF16)
            ash = spool.tile([KC, BL * S], F32)
            alsc = spool.tile([BL, S], F32)

            nc.gpsimd.memset(h_b[:], 0.0)
            nc.gpsimd.memset(hT[:], 0.0)
            nc.gpsimd.memset(alf[:], 0.0)
            nc.gpsimd.memset(trans[:], 0.5)
            nc.gpsimd.memset(aD[:], 0.0)
            nc.gpsimd.memset(ash[:], 0.0)
            nc.sync.dma_start(apd.ap()[:, :], ash[0:BL, 0:PAD])
            nc.gpsimd.memset(alf[:, 1:2], 1.0)
            for b in range(BL):
                nc.gpsimd.memset(aD[0:1, 2 * b, b:b + 1], 1.0)
            nc.gpsimd.memset(alsc[:], 1.0 / S)
            nc.sync.dma_start(apd.ap()[:, 15:15 + S], alsc[:])
            nc.sync.dma_start(ash[:, :],
                              _ap(apd, [[1, KC], [PAD, BL], [1, S]]))

            def ctx_block():
                cps = ppool.tile([BL, I], F32, tag="pB")
                for kc in range(8):
                    nc.tensor.matmul(cps[:], aD[:, kc, :], enc_bf[:, kc, :],
                                     start=(kc == 0), stop=(kc == 7))
                ctx_b = wpool.tile([BL, I], F32, tag="ctxb")
                nc.vector.tensor_copy(ctx_b[:], cps[:])
                tps = ppool.tile([128, 4, BL], F32, tag="pB")
                for ck in range(4):
                    nc.tensor.transpose(
                        tps[:, ck, :], ctx_b[:, ck * 128:(ck + 1) * 128],
                        id4[:])
                nc.vector.tensor_copy(ctxT[:], tps[:])

            ctx_block()

            # ================= scan =================
            with tc.For_i(0, R, BL, hint_engines=(PE,),
                          staggered_reset=True) as iv:
                # ---- gate preactivations ----
                # gx_rz: [4, 2048] = (gq+bias) + gprev + gh  (r,z cols)
                # gn2:   [4, 2048] = [xn | ghn]
                gxrz = ppool.tile([BL, 2 * C], F32, tag="pA")
                gn2 = ppool.tile([BL, 2 * C], F32, tag="pB")

                gq_sb = wpool.tile([BL, G3], BF16, tag="gqstep")
                nc.sync.dma_start(gq_sb[:], gq_d.ap()[bass.ds(iv, BL), :])

                def gdst(nk):
                    if nk < 4:
                        return gxrz[:, nk * 512:(nk + 1) * 512]
                    return gn2[:, (nk - 2) * 512:(nk - 1) * 512]  # ghn half

                for kc in range(8):  # gh
                    for nk in range(6):
                        nc.tensor.matmul(
                            gdst(nk), hT[:, kc, :],
                            whh[:, kc, nk * 512:(nk + 1) * 512],
                            start=(kc == 0), stop=False)
                for nk in range(2):  # b_hh_n closes ghn
                    nc.tensor.matmul(
                        gn2[:, (nk + 2) * 512:(nk + 3) * 512],
                        ones1[:, :BL],
                        bhhn[:, nk * 512:(nk + 1) * 512],
                        start=False, stop=True)

                def xdst(nk):
                    if nk < 4:
                        return gxrz[:, nk * 512:(nk + 1) * 512]
                    return gn2[:, (nk - 4) * 512:(nk - 3) * 512]  # xn half

                for kc in range(4):  # gprev
                    for nk in range(6):
                        nc.tensor.matmul(
                            xdst(nk), ctxT[:, kc, :],
                            wihp[:, kc, nk * 512:(nk + 1) * 512],
                            start=(kc == 0 and nk >= 4), stop=False)
                for nk in range(6):  # gq identity-add closes gxrz + xn
                    nc.tensor.matmul(
                        xdst(nk), i4bf[:],
                        gq_sb[:, nk * 512:(nk + 1) * 512],
                        start=False, stop=True)

                # ---- gates / GRU update (row layout) ----
                trz = wpool.tile([BL, 2 * C], F32, tag="trz")
                nc.scalar.activation(trz[:], gxrz[:], AF.Tanh, scale=0.5)
                hn05 = wpool.tile([BL, C], F32, tag="gtmp", bufs=2)
                nc.scalar.activation(hn05[:], gn2[:, C:], AF.Copy, scale=0.5)
                o2 = wpool.tile([BL, C], F32, tag="gtmp", bufs=2)
                nc.vector.scalar_tensor_tensor(
                    o2[:], trz[:, :C], 1.0, hn05[:], ALU.add, ALU.mult)
                narg = wpool.tile([BL, C], F32, tag="gtmp", bufs=2)
                nc.vector.tensor_add(narg[:], gn2[:, :C], o2[:])
                ngate = wpool.tile([BL, C], F32, tag="ngate")
                nc.scalar.activation(ngate[:], narg[:], AF.Tanh)
                dmn = wpool.tile([BL, C], F32, tag="gtmp", bufs=2)
                nc.vector.tensor_sub(dmn[:], h_b[:], ngate[:])
                o5 = wpool.tile([BL, C], F32, tag="gtmp", bufs=2)
                nc.vector.scalar_tensor_tensor(
                    o5[:], trz[:, C:], 1.0, dmn[:], ALU.add, ALU.mult)
                nc.vector.scalar_tensor_tensor(
                    h_b[:], o5[:], 0.5, ngate[:], ALU.mult, ALU.add)

                hps = ppool.tile([128, 8, BL], F32, tag="pB")
                for ck in range(8):
                    nc.tensor.transpose(
                        hps[:, ck, :], h_b[:, ck * 128:(ck + 1) * 128], id4[:])
                nc.vector.tensor_copy(hT[:], hps[:])

                # ---- qp = h_new @ w_q.T ----
                qps = ppool.tile([BL, C2], F32, tag="pB")
                for kc in range(8):
                    nc.tensor.matmul(qps[:], hT[:, kc, :], wq[:, kc, :],
                                     start=(kc == 0), stop=(kc == 7))
                qp_sb = wpool.tile([BL, C2], F32, tag="qpsb")
                nc.vector.tensor_copy(qp_sb[:], qps[:])

                # ---- score (two 2-chunk passes) + tanh + energy ----
                eps = ppool.tile([1, BL * S], F32, tag="pB")
                for hp in range(2):
                    scps = ppool.tile([128, 2, BL * S], F32, tag="pA")
                    for m2 in range(2):
                        mc = hp * 2 + m2
                        for nk in range(2):
                            sl = scps[:, m2, nk * 512:(nk + 1) * 512]
                            nc.tensor.matmul(
                                sl, weff[:, mc * 128:(mc + 1) * 128],
                                ash[:, nk * 512:(nk + 1) * 512],
                                start=True, stop=False)
                            nc.tensor.matmul(
                                sl, qp_sb[:, mc * 128:(mc + 1) * 128],
                                bsel[:, nk * 512:(nk + 1) * 512],
                                start=False, stop=True)
                    for m2 in range(2):
                        mc = hp * 2 + m2
                        ssbt = wpool.tile([128, BL * S], F32, tag="ssbt",
                                          bufs=1)
                        nc.vector.scalar_tensor_tensor(
                            ssbt[:], scps[:, m2, :], bk[:, mc:mc + 1],
                            key_sb[:, mc, :], ALU.add, ALU.add)
                        taut = wpool.tile([128, BL * S], BF16, tag="taut",
                                          bufs=1)
                        nc.scalar.activation(taut[:], ssbt[:], AF.Tanh)
                        for nk in range(2):
                            nc.tensor.matmul(
                                eps[:, nk * 512:(nk + 1) * 512],
                                wagg[:, mc:mc + 1],
                                taut[:, nk * 512:(nk + 1) * 512],
                                start=(mc == 0), stop=(mc == 3))

                # ---- softmax / alpha recursion ----
                erow = wpool.tile([1, BL * S], F32, tag="ssbt", bufs=1)
                nc.scalar.activation(erow[:], eps[:], AF.Exp)
                nc.sync.dma_start(eb_d.ap()[:], erow[:])
                e4 = wpool.tile([BL, S], F32, tag="e4")
                nc.sync.dma_start(e4[:], _ap(eb_d, [[S, BL], [1, S]]))

                omt = wpool.tile([BL, 1], F32, tag="omt")
                nc.vector.tensor_scalar(omt[:], trans[:], -1.0, 1.0,
                                        ALU.mult, ALU.add)
                m1 = wpool.tile([BL, S], F32, tag="al", bufs=2)
                nc.vector.tensor_scalar(m1[:], alf[:, 1:], omt[:], 1e-7,
                                        ALU.mult, ALU.add)
                mix = wpool.tile([BL, S], F32, tag="al", bufs=2)
                nc.vector.scalar_tensor_tensor(
                    mix[:], alf[:, 0:S], trans[:], m1[:], ALU.mult, ALU.add)
                u = wpool.tile([BL, S], F32, tag="al", bufs=2)
                nc.vector.tensor_mul(u[:], mix[:], e4[:])
                usum = wpool.tile([BL, 1], F32, tag="usum")
                nc.vector.reduce_sum(usum[:], u[:], mybir.AxisListType.X)
                urec = wpool.tile([BL, 1], F32, tag="urec")
                nc.vector.reciprocal(urec[:], usum[:])
                nc.vector.tensor_scalar(alf[:, 1:], u[:], urec[:], None,
                                        ALU.mult)
                nc.sync.dma_start(alphas_d.ap()[bass.ds(iv, BL), :],
                                  alf[:, 1:])

                # align for next step's conv
                zs = wpool.tile([BL, 1], F32, tag="zs")
                nc.vector.reduce_sum(zs[:], e4[:], mybir.AxisListType.X)
                zr = wpool.tile([BL, 1], F32, tag="zr")
                nc.vector.reciprocal(zr[:], zs[:])
                nc.vector.tensor_scalar(alsc[:], e4[:], zr[:], None, ALU.mult)
                nc.sync.dma_start(apd.ap()[:, 15:15 + S], alsc[:])
                nc.sync.dma_start(ash[:, :],
                                  _ap(apd, [[1, KC], [PAD, BL], [1, S]]))

                # ---- alpha -> aD (block diagonal, bf16) ----
                aps = ppool.tile([128, 2, BL], F32, tag="pB")
                nc.tensor.transpose(aps[:, 0, :], alf[:, 1:129], id4[:])
                nc.tensor.transpose(aps[:, 1, :], alf[:, 129:257], id4[:])
                for seg in range(2):
                    dst = _ap(aD[:], [[8 * BL, 128], [2 * BL + 1, BL]],
                              BL * seg)
                    nc.vector.tensor_copy(dst, aps[:, seg, :])

                # ---- ctx (= attend_t = prev_{t+1}) ----
                ctx_block()

                # ---- t-branch: trans_{t+1} ----
                t1p = ppool.tile([BL, C], F32, tag="pB")
                for nk in range(2):
                    cs = slice(nk * 512, (nk + 1) * 512)
                    for kc in range(4):
                        nc.tensor.matmul(
                            t1p[:, cs], ctxT[:, kc, :], wt1a[:, kc, cs],
                            start=(kc == 0), stop=False)
                    for kc in range(8):
                        nc.tensor.matmul(
                            t1p[:, cs], hT[:, kc, :], wt1h[:, kc, cs],
                            start=False, stop=False)
                frt = wpool.tile([M, BL], BF16, tag="frt")
                nc.sync.dma_start(frt[:], frT_d.ap()[bass.ds(iv, BL), :, :])
                for nk in range(2):
                    cs = slice(nk * 512, (nk + 1) * 512)
                    nc.tensor.matmul(t1p[:, cs], frt[:], wt1f[:, cs],
                                     start=False, stop=False)
                    nc.tensor.matmul(t1p[:, cs], ones4bf[:], bt1[:, cs],
                                     start=False, stop=True)
                tt1 = wpool.tile([BL, C], F32, tag="gtmp", bufs=2)
                nc.scalar.activation(tt1[:], t1p[:], AF.Tanh)
                tu = wpool.tile([BL, C], F32, tag="gtmp", bufs=2)
                nc.vector.tensor_mul(tu[:], tt1[:], wt2r[:])
                ts = wpool.tile([BL, 1], F32, tag="ts")
                nc.vector.reduce_sum(ts[:], tu[:], mybir.AxisListType.X)
                tt = wpool.tile([BL, 1], F32, tag="tt")
                nc.scalar.activation(tt[:], ts[:], AF.Tanh, scale=0.5)
                nc.vector.tensor_scalar(trans[:], tt[:], 0.5, 0.5,
                                        ALU.mult, ALU.add)

    return nc


def _prep_shared(inputs):
    w_ih = np.asarray(inputs["w_ih"], np.float32)
    w_hh = np.asarray(inputs["w_hh"], np.float32)
    b_ih = np.asarray(inputs["b_ih"], np.float32)
    b_hh = np.asarray(inputs["b_hh"], np.float32)
    w_q = np.asarray(inputs["w_q"], np.float32)
    w_loc1 = np.asarray(inputs["w_loc1"], np.float32)
    w_loc2 = np.asarray(inputs["w_loc2"], np.float32)
    w_k = np.asarray(inputs["w_k"], np.float32)
    b_k = np.asarray(inputs["b_k"], np.float32)
    w_agg = np.asarray(inputs["w_agg"], np.float32)
    w_t1 = np.asarray(inputs["w_t1"], np.float32)
    b_t1 = np.asarray(inputs["b_t1"], np.float32)
    w_t2 = np.asarray(inputs["w_t2"], np.float32)

    w_eff = w_loc2 @ w_loc1[:, 0, :]  # [C2, KC]
    bias1 = b_ih + np.concatenate([b_hh[:2 * C], np.zeros(C, np.float32)])
    bsel = np.zeros((BL, BL * S), np.float32)
    for b in range(BL):
        bsel[b, b * S:(b + 1) * S] = 1.0

    cc = np.ascontiguousarray

    def chunk(a):  # [nk*128, X] -> [128, nk, X]
        nk = a.shape[0] // 128
        return cc(a.reshape(nk, 128, -1).transpose(1, 0, 2))

    return {
        "wihp": chunk(w_ih[:, H:].T),
        "whh": chunk(w_hh.T),
        "wq": chunk(w_q.T),
        "wt1a": chunk(w_t1[:, :I].T),
        "wt1h": chunk(w_t1[:, I + M:].T),
        "wt1f": cc(w_t1[:, I:I + M].T),
        "wihq": chunk(w_ih[:, :H].T),
        "wk": chunk(w_k.T),
        "weff": cc(w_eff.T),
        "wagg": cc(w_agg.reshape(4, 128).T),
        "bk": cc(b_k.reshape(4, 128).T),
        "bias1": bias1.reshape(1, G3),
        "bhhn": cc(b_hh[2 * C:].reshape(1, C)),
        "bt1": cc(b_t1.reshape(1, C)),
        "bsel": bsel,
        "i4bf": np.eye(BL, dtype=np.float32),
        "ones4bf": np.ones((1, BL), np.float32),
        "id4": np.eye(BL, dtype=np.float32),
        "ones1": np.ones((1, 128), np.float32),
        "wt2r": np.tile(w_t2.reshape(1, C), (BL, 1)),
    }


_BF16_NAMES = {"enc_bf", "qT", "frT", "wihp", "whh", "wq", "wt1a", "wt1h",
               "wt1f", "wihq", "wagg", "bt1", "i4bf", "ones4bf"}


def make_in_maps(inputs):
    import ml_dtypes

    def cast(name, arr):
        if name in _BF16_NAMES:
            return np.asarray(arr, np.float32).astype(ml_dtypes.bfloat16)
        return np.ascontiguousarray(arr, np.float32)

    T = inputs["queries"].shape[1]
    shared = _prep_shared(inputs)
    enc = np.asarray(inputs["encodings"], np.float32)
    qs = np.asarray(inputs["queries"], np.float32)
    outs = np.asarray(inputs["outputs"], np.float32)

    in_maps = []
    for c in range(NCORES):
        sl = slice(c * BL, (c + 1) * BL)
        e = enc[sl].reshape(BL * S, I)
        q = qs[sl]
        fr = outs[sl]
        m = {k: cast(k, v) for k, v in shared.items()}
        m["enc_bf"] = cast("enc_bf", e.reshape(8, 128, I).transpose(1, 0, 2))
        m["encT"] = cast("encT", e.T.reshape(4, 128, BL * S).transpose(1, 0, 2))
        m["qT"] = cast("qT", q.transpose(2, 1, 0).reshape(
            2, 128, T * BL).transpose(1, 0, 2))
        m["frT"] = cast("frT", fr.transpose(1, 2, 0).reshape(
            T, BL, M // BL, BL).reshape(T * BL, M // BL, BL))
        in_maps.append(m)
    return in_maps


def kernel(**inputs):
    mask = np.asarray(inputs["mask"])
    assert np.all(mask == 1.0), "kernel assumes all-ones mask"
    T = inputs["queries"].shape[1]

    import os, time as _time
    in_maps = make_in_maps(inputs)
    nc = build_program(T)
    nc.compile()
    t0 = _time.time()
    res = run_bass_kernel_spmd(nc, in_maps, list(range(NCORES)))
    if os.environ.get("ALIGNER_BENCH"):
        print(f"exec+jit wall: {_time.time()-t0:.2f}s", flush=True)
        for it in range(2):
            t0 = _time.time()
            res = run_bass_kernel_spmd(nc, in_maps, list(range(NCORES)))
            w = _time.time() - t0
            print(f"exec wall[{it}]: {w:.3f}s  HW exec time: {w*1e9:.0f} ns",
                  flush=True)
    out = np.zeros((B_FULL, T, S), np.float32)
    for c in range(NCORES):
        a = np.asarray(res.results[c]["alphas"], np.float32).reshape(T, BL, S)
        out[c * BL:(c + 1) * BL] = a.transpose(1, 0, 2)
    return out


if __name__ == "__main__":
    build_program(2)
    print("build ok")
